# revision 27
# baseline (speedup 1.0000x reference)
"""Multi-head self-attention (QK^T -> softmax -> ctx -> linear) on 8 TRN2 cores.

Sharding: each core owns one (batch, query-block) shard: batch = core//4,
queries [qlo, qlo+512) with qlo = (core%4)*512. Attention needs all keys of
the core's batch, so keys are replicated per batch; no collectives needed.

Per core (head h, its 512 queries q, all 2048 keys k), all operands bf16:
  S_T[k, q]  = sum_d x[k, hd+d] * x[q, hd+d]            (PE, psum f32)
  P_T[k, q]  = exp(0.125 * S_T[k, q])                   (ACT, psum->sbuf bf16)
  ctx[q, m]  = sum_k P_T[k, q] * xa[k, m]               (PE; P parked as the
               stationary operand so only the 65-wide xa side streams;
               m = 64 dims + ones column -> softmax denominator at m=64)
  chunk[q,i] = ctx[q, d] / ctx[q, 64]                   (DVE tensor_scalar,
               per-partition recip scalar; -> bf16)
  chunkT     = transpose(chunk)                         (DMA xbar transpose)
  out[q, o]  = sum_i chunkT[i, q] * W[o, i] + b[o]      (PE + DVE adds)

The PE cost model charges only streamed output columns (stationary loads are
free), so parking P halves ctx cost vs streaming it; everything else is
orientation-chosen to keep output partitions full.
"""

import sys

for _p in ("/opt/trn_rl_repo", "/root/.axon_site/_ro/trn_rl_repo"):
    if _p not in sys.path:
        sys.path.append(_p)

import numpy as np

import concourse.bacc as bacc
import concourse.bass as bass
import concourse.mybir as mybir
import concourse.tile as tile

F32 = mybir.dt.float32
BF16 = mybir.dt.bfloat16

B, L, H, NH, DH = 2, 2048, 1024, 16, 64
NCORES = 8
QB = 512
KBLKS = L // 128          # 16 key blocks
NPAIR = NH // 2           # 8 head pairs
AUG = DH + 1              # 64 dims + ones column
HC = H // 128             # 8 hidden chunks (one per pair)
OBW = 512                 # proj output column block
SCALE = float(1.0 / np.sqrt(DH))
QSUB = QB // 128          # 4 query subtiles


def build_nc():
    nc = bacc.Bacc("TRN2")
    xt = nc.declare_dram_parameter("xt", [NPAIR, 128, L], BF16, isOutput=False)
    xq = nc.declare_dram_parameter("xq", [NPAIR, 128, QB], BF16, isOutput=False)
    xa = nc.declare_dram_parameter("xa", [NPAIR, 128, 2 * KBLKS * AUG], BF16, isOutput=False)
    wt = nc.declare_dram_parameter("wt", [128, HC * H], BF16, isOutput=False)
    bias = nc.declare_dram_parameter("bias", [1, H], BF16, isOutput=False)
    ones = nc.declare_dram_parameter("ones", [1, 128], BF16, isOutput=False)
    ident = nc.declare_dram_parameter("ident", [128, 128], BF16, isOutput=False)
    out = nc.declare_dram_parameter("out", [QB, H], F32, isOutput=True)

    NSTEP = NPAIR * KBLKS
    SKEW = 2
    # scheduling floor per global step (ns): keeps the tile scheduler from
    # hoisting proj work into much earlier PE positions, where an unmet
    # transpose dep would stall the PE counter that gates the exp stream.
    TSTEP_NS = 1040.0
    TBASE_NS = 4600.0

    def floor_ms(gs):
        return (TBASE_NS + gs * TSTEP_NS) / 1e6

    # Schraudolph-pair constants (bf16 domain): t = scores*SCALE*log2(e);
    # int16 = t*128 + 127*128 + C, with C folding the /2 normalization
    EXPA = float(128.0 * SCALE * 1.4426950408889634)
    # +22.375: scale calibration so the equal-weight pair averages to 1.0x
    # of true exp (matching the ACT-exp key blocks in the same softmax)
    EXPB1 = float(127 * 128 - 128 + 22.375)
    EXPB2 = float(127 * 128 - 192 + 22.375)
    # offload in runs of 3 so the psum-slot refill chain is amortized once
    # per run instead of once per step
    OFF_STARTS = globals().get('_OFF_STARTS', ())
    OFFLOAD = frozenset(
        g for start in OFF_STARTS for g in (start, start + 1, start + 2)
    )
    OFF_SKEW = 5

    with tile.TileContext(nc) as tc:
        with (
            tc.tile_pool(name="xt", bufs=2) as xt_pool,
            tc.tile_pool(name="xq", bufs=2) as xq_pool,
            tc.tile_pool(name="xa", bufs=2) as xa_pool,
            tc.tile_pool(name="p", bufs=10) as p_pool,
            tc.tile_pool(name="cq", bufs=8) as cq_pool,
            tc.tile_pool(name="rc", bufs=4) as rc_pool,
            tc.tile_pool(name="y", bufs=4) as y_pool,
            tc.tile_pool(name="consts", bufs=1) as consts,
            tc.tile_pool(name="spsum", bufs=2, space="PSUM") as s_psum,
            tc.tile_pool(name="ctxpsum", bufs=3, space="PSUM") as ctx_psum,
            tc.tile_pool(name="prjpsum", bufs=1, space="PSUM") as prj_psum,
        ):
            ones_t = consts.tile([1, 128], BF16)
            ident_t = consts.tile([128, 128], BF16)
            bias_t = consts.tile([1, H], BF16)
            bias_bc = consts.tile([128, H], F32)

            wt_ts = [
                consts.tile([128, H], BF16, tag=f"wt{c}", name=f"wt{c}")
                for c in range(HC)
            ]
            chunks = [
                consts.tile([128, QB], BF16, tag=f"ch{c}", name=f"ch{c}")
                for c in range(HC)
            ]
            acc = [
                consts.tile([128, H], F32, tag=f"acc{q}", name=f"acc{q}")
                for q in range(QSUB)
            ]

            def pair_dmas(pr, split_xt=False):
                xt_t = xt_pool.tile([128, L], BF16, tag="xt", name=f"xt{pr}")
                xq_t = xq_pool.tile([128, QB], BF16, tag="xq", name=f"xq{pr}")
                nc.sync.dma_start(xq_t[:], xq[pr])
                if split_xt:
                    # first key quarter lands fast so scores can start early
                    nc.sync.dma_start(xt_t[:, 0:512], xt[pr][:, 0:512])
                    nc.sync.dma_start(xt_t[:, 512:L], xt[pr][:, 512:L])
                else:
                    nc.sync.dma_start(xt_t[:], xt[pr])
                xa_t = xa_pool.tile([128, 2 * KBLKS * AUG], BF16, tag="xa", name=f"xa{pr}")
                nc.sync.dma_start(xa_t[:], xa[pr])
                return [xt_t, xq_t, xa_t, None, None]

            def emit_proj_group(clist, qs, ob, pool, tag, eng=None):
                obsl = slice(ob * OBW, (ob + 1) * OBW)
                qsl = slice(qs * 128, (qs + 1) * 128)
                cp = pool.tile([128, 2 * QB] if tag == "s" else [128, OBW],
                               F32, tag=tag, name=f"cp{clist[0]}_{qs}_{ob}")
                for i, c in enumerate(clist):
                    nc.tensor.matmul(
                        cp[:, 0:OBW], chunks[c][:, qsl], wt_ts[c][:, obsl],
                        start=(i == 0), stop=(i == len(clist) - 1),
                    )
                prev = bias_bc if clist[0] == 0 else acc[qs]
                (eng or nc.vector).tensor_add(
                    acc[qs][:, obsl], cp[:, 0:OBW], prev[:, obsl]
                )

            def emit_drain(prp, ctx_a, ctx_b):
                # normalize pair prp's ctx accumulators and transpose into
                # the proj chunk tile; for the final pair, pipeline the
                # remaining projection groups and output DMAs per q-subtile
                last = prp == NPAIR - 1
                rc_a = rc_pool.tile([128, QSUB], F32, tag="rc", name=f"rca{prp}")
                rc_b = rc_pool.tile([128, QSUB], F32, tag="rc", name=f"rcb{prp}")
                with nc.allow_low_precision(reason="softmax denominator recip"):
                    nc.vector.reciprocal(rc_a[:], ctx_a[:, DH : QSUB * AUG : AUG])
                    nc.vector.reciprocal(rc_b[:], ctx_b[:, DH : QSUB * AUG : AUG])
                for qs in range(QSUB):
                    cq = cq_pool.tile([128, 128], BF16, tag="cq", name=f"cq{prp}_{qs}")
                    nc.vector.tensor_scalar_mul(
                        cq[:, 0:DH], ctx_a[:, qs * AUG : qs * AUG + DH],
                        rc_a[:, qs : qs + 1],
                    )
                    nc.vector.tensor_scalar_mul(
                        cq[:, DH : 2 * DH], ctx_b[:, qs * AUG : qs * AUG + DH],
                        rc_b[:, qs : qs + 1],
                    )
                    if not last:
                        nc.sync.dma_start_transpose(
                            chunks[prp][:, qs * 128 : (qs + 1) * 128], cq[:, :]
                        )
                        continue
                    # final pair: PE-transpose (HWDGE is slow to drain) + ACT
                    # copy; fused (5,6,7) groups follow in a second pass so
                    # the DVE adds overlap later q-subtiles' normalizes
                    tp = s_psum.tile([128, 128], BF16, tag="s", name=f"tp{qs}")
                    nc.tensor.transpose(tp[:], cq[:, :], ident_t[:])
                    qsl = slice(qs * 128, (qs + 1) * 128)
                    nc.scalar.copy(chunks[prp][:, qsl], tp[:])
                if last:
                    for qs in range(QSUB):
                        qsl = slice(qs * 128, (qs + 1) * 128)
                        clist = (5, 6, 7) if qs == 0 else (7,)
                        cp = s_psum.tile([128, 2 * QB], F32, tag="s", name=f"tcp{qs}")
                        for ob in range(2):
                            obsl = slice(ob * OBW, (ob + 1) * OBW)
                            for i, c in enumerate(clist):
                                nc.tensor.matmul(
                                    cp[:, obsl], chunks[c][:, qsl], wt_ts[c][:, obsl],
                                    start=(i == 0), stop=(i == len(clist) - 1),
                                )
                        if qs < QSUB - 1:
                            nc.vector.tensor_add(acc[qs][:], cp[:], acc[qs][:])
                            nc.sync.dma_start(out[qsl, :], acc[qs][:])
                        else:
                            # quarter-split so the final DMA transfer is small
                            for q4 in range(4):
                                sl = slice(q4 * 256, (q4 + 1) * 256)
                                nc.vector.tensor_add(
                                    acc[qs][:, sl], cp[:, sl], acc[qs][:, sl]
                                )
                                nc.sync.dma_start(out[qsl, sl], acc[qs][:, sl])

            # proj emission plan: step -> (chunk list, qs, ob)
            # chunk-pairs (0,1)@pairs2-3, (2,3)@pairs4-5; chunk 4 singles@pair6,
            # (5,6)@pair7, chunk 7 + leftovers in the tail.
            # window steps start ~4 kbs after the newer chunk's drain so the
            # first group never waits on an in-flight transpose
            plan = {}
            for hp, base_pr in ((0, 2), (1, 4)):
                cl = [2 * hp, 2 * hp + 1]
                steps = [base_pr * KBLKS + k for k in (6, 8, 10, 12, 14)] + [
                    (base_pr + 1) * KBLKS + k for k in (4, 8, 12)
                ]
                for gidx, st in enumerate(steps):
                    qs, ob = divmod(gidx, 2)
                    plan[st] = (cl, qs, ob)
            for i, kbw in enumerate((3, 5, 7, 9, 11, 13, 15)):
                plan[6 * KBLKS + kbw] = ([4], i // 2, i % 2)
            plan[7 * KBLKS + 1] = ([4], 3, 1)
            for i, kbw in enumerate((5, 7, 9, 11, 13, 15)):
                plan[7 * KBLKS + kbw] = ([5, 6], 1 + i // 2, i % 2)

            tiles = {}
            pipe = []
            y1s = {}
            diag_p = {}
            for gs in range(NSTEP + OFF_SKEW):
                cur = None
                if gs < NSTEP:
                    pr, kb = divmod(gs, KBLKS)
                    if pr == 0 and kb == 0:
                        warm = consts.tile([1, 512], BF16)
                        nc.vector.memset(warm[:], 1.0)
                        for wi in range(8):
                            wps = prj_psum.tile([128, OBW], F32, tag="prj",
                                                name=f"warm{wi}")
                            nc.tensor.matmul(
                                wps[:], warm[0:1, 0:128], warm[0:1, :],
                                start=True, stop=True,
                            )
                        tiles[0] = pair_dmas(0, split_xt=True)
                        nc.sync.dma_start(wt_ts[0][:], wt[:, 0:H])
                        nc.sync.dma_start(ones_t[:], ones[:])
                        nc.sync.dma_start(bias_t[:], bias[:])
                        nc.sync.dma_start(ident_t[:], ident[:])
                    if pr == 0 and kb == 6:
                        # bias broadcast across partitions via K=1 ones matmul
                        # (uses the proj psum bank, idle until pair 2, so the
                        # late-landing bias DMA never stalls the score slots)
                        for ob in range(H // OBW):
                            obsl = slice(ob * OBW, (ob + 1) * OBW)
                            bps = prj_psum.tile([128, OBW], F32, tag="prj")
                            nc.tensor.matmul(
                                bps[:], ones_t[0:1, :], bias_t[0:1, obsl],
                                start=True, stop=True,
                            )
                            nc.vector.tensor_copy(bias_bc[:, obsl], bps[:])
                    if kb == 2 and pr + 1 < NPAIR:
                        # floor keeps prefetches behind the previous
                        # drain's transposes in the HWDGE queue
                        with tc.tile_wait_until(floor_ms((pr + 1) * KBLKS - 6)):
                            tiles[pr + 1] = pair_dmas(pr + 1)
                    if kb == 4 and pr + 1 < NPAIR:
                        with tc.tile_wait_until(floor_ms(min((pr + 3) * KBLKS - 10, NSTEP))):
                            nc.sync.dma_start(
                                wt_ts[pr + 1][:], wt[:, (pr + 1) * H : (pr + 2) * H]
                            )
                    xt_t, xq_t, xa_t, _, _ = tiles[pr]
                    ksl = slice(kb * 128, (kb + 1) * 128)
                    # keys are permuted per core so kb 0-3 is the core's own
                    # query block: S(i,j) = S(j,i)^T there, so compute only
                    # q-subtiles j <= kb and fill the rest by transposing
                    # later steps' exp'd blocks (exact, on idle DMA engines)
                    dw = (kb + 1) * 128 if kb < QSUB else QB
                    s_ab = s_psum.tile([128, 2 * QB], F32, tag="s")
                    nc.tensor.matmul(
                        s_ab[:, 0:dw], xt_t[0:64, ksl], xq_t[0:64, 0:dw],
                        start=True, stop=True,
                    )
                    nc.tensor.matmul(
                        s_ab[:, QB : QB + dw], xt_t[64:128, ksl],
                        xq_t[64:128, 0:dw],
                        start=True, stop=True,
                    )
                    if gs in OFFLOAD:
                        # bit-trick exp, step 1 of 2: y1 = int16(s*A + B1).
                        # This is the only psum reader, so the score slot is
                        # released at ACT-parity latency; the combine runs two
                        # steps later (see the finish block below).
                        y1 = y_pool.tile([128, 2 * QB], mybir.dt.int16,
                                         tag="y1", name=f"y1_{gs}")
                        nc.vector.tensor_scalar(
                            y1[:], s_ab[:], EXPA, EXPB1,
                            op0=mybir.AluOpType.mult, op1=mybir.AluOpType.add,
                        )
                        y1s[gs] = y1
                        cur = (gs, None)
                    else:
                        p_ab = p_pool.tile([128, 2 * QB], BF16, tag="p")
                        if kb < QSUB:
                            sv = s_ab.rearrange("p (h q) -> p h q", h=2)
                            pv = p_ab.rearrange("p (h q) -> p h q", h=2)
                            nc.scalar.activation(
                                pv[:, :, 0:dw], sv[:, :, 0:dw],
                                mybir.ActivationFunctionType.Exp, scale=SCALE,
                            )
                            diag_p[kb] = p_ab
                            for i in range(kb):
                                for hh in range(2):
                                    nc.sync.dma_start_transpose(
                                        diag_p[i][:, hh * QB + kb * 128
                                                  : hh * QB + (kb + 1) * 128],
                                        p_ab[:, hh * QB + i * 128
                                             : hh * QB + (i + 1) * 128],
                                    )
                        else:
                            nc.scalar.activation(
                                p_ab[:], s_ab[:],
                                mybir.ActivationFunctionType.Exp, scale=SCALE,
                            )
                        cur = (gs, p_ab)
                if cur is not None:
                    pipe.append(cur)
                tfin = gs - 2
                if tfin in OFFLOAD:
                    # bit-trick exp, step 2: P = S(B1) + S(B1-64); the pair
                    # cancels the interpolation ripple to ~0.6% rms and
                    # round(t-64) == round(t)-64 exactly -> int16 subtract
                    y1 = y1s.pop(tfin)
                    y2 = y_pool.tile([128, 2 * QB], mybir.dt.int16,
                                     tag="y2", name=f"y2_{tfin}")
                    nc.vector.tensor_scalar_sub(y2[:], y1[:], 64)
                    p_fin = p_pool.tile([128, 2 * QB], BF16, tag="p",
                                        name=f"pf{tfin}")
                    nc.vector.tensor_add(
                        p_fin[:], y2[:].bitcast(BF16), y1[:].bitcast(BF16)
                    )
                    for pi, (pgs, pv) in enumerate(pipe):
                        if pgs == tfin:
                            pipe[pi] = (pgs, p_fin)
                            break
                prevs = []
                while pipe and len(prevs) < 2:
                    hkb = pipe[0][0] % KBLKS
                    need = OFF_SKEW if pipe[0][0] in OFFLOAD else (
                        max(SKEW, 6 - hkb) if hkb < QSUB else SKEW
                    )
                    if gs - pipe[0][0] >= need or gs >= NSTEP + SKEW - 1:
                        prevs.append(pipe.pop(0))
                    else:
                        break
                for prev in prevs:
                    gsp, pp = prev
                    prp, kbp = divmod(gsp, KBLKS)
                    if kbp == 0:
                        ctx_a = ctx_psum.tile([128, QSUB * AUG], F32, tag="ctx",
                                              name=f"ctxa{prp}")
                        ctx_b = ctx_psum.tile([128, QSUB * AUG], F32, tag="ctx",
                                              name=f"ctxb{prp}")
                        tiles[prp][3] = ctx_a
                        tiles[prp][4] = ctx_b
                    _, _, xa_t, ctx_a, ctx_b = tiles[prp]
                    for j, ctx_t in ((0, ctx_a), (1, ctx_b)):
                        asl = slice(j * KBLKS * AUG + kbp * AUG,
                                    j * KBLKS * AUG + (kbp + 1) * AUG)
                        for qs in range(QSUB):
                            # The 4 qs accumulation groups share one psum
                            # bank (= one 2KB zero region): only the tile's
                            # first matmul may set start (start marks the
                            # WHOLE region pending-zero, so later groups'
                            # first writes still overwrite-not-accumulate),
                            # and only the last sets stop.
                            nc.tensor.matmul(
                                ctx_t[:, qs * AUG : (qs + 1) * AUG],
                                pp[:, j * QB + qs * 128 : j * QB + (qs + 1) * 128],
                                xa_t[:, asl],
                                start=(kbp == 0 and qs == 0),
                                stop=(kbp == KBLKS - 1 and qs == QSUB - 1),
                            )
                    g = plan.get(gsp)
                    if g is not None:
                        with tc.tile_wait_until(floor_ms(gsp)):
                            emit_proj_group(g[0], g[1], g[2], prj_psum, "prj")
                    if kbp == KBLKS - 1:
                        emit_drain(prp, ctx_a, ctx_b)
    nc.compile()
    return nc


def _to_bf16(x):
    import ml_dtypes

    return np.asarray(x, np.float32).astype(ml_dtypes.bfloat16)


def shard_inputs(key, W_ctx, b_ctx):
    """Host-side prep of per-core input dicts (bf16 layouts).

    Keys are permuted per core so the core's own query block comes first:
    softmax is key-order invariant, and this puts the symmetric diagonal
    score block at kb 0..3 identically on every core (SPMD-friendly)."""
    Bv = key.shape[0]
    cores_per_batch = NCORES // Bv

    key = np.asarray(key, dtype=np.float32)
    wt_host = np.ascontiguousarray(
        np.asarray(W_ctx, np.float32).T.reshape(HC, 128, H).transpose(1, 0, 2)
        .reshape(128, HC * H)
    )
    bias_host = np.asarray(b_ctx, np.float32).reshape(1, H)
    wt_b = _to_bf16(wt_host)
    bias_b = _to_bf16(bias_host)
    ones_b = _to_bf16(np.ones((1, 128), dtype=np.float32))
    ident_b = _to_bf16(np.eye(128, dtype=np.float32))

    in_maps = []
    meta = []
    for c in range(NCORES):
        b = c // cores_per_batch
        qlo = (c % cores_per_batch) * QB
        xp = np.concatenate(
            [key[b, qlo : qlo + QB], key[b, :qlo], key[b, qlo + QB :]], axis=0
        )
        xh = xp.reshape(L, NH, DH)
        # xt: [NPAIR, 128, L]; pair p rows 0:64 = head 2p, 64:128 = head 2p+1
        xt_full = np.ascontiguousarray(
            xh.transpose(1, 2, 0).reshape(NPAIR, 2 * DH, L)
        )
        # xa: [NH, 128, KBLKS*AUG] with ones at kb*AUG+DH, then pair-merged
        xa_full = np.empty((NH, 128, KBLKS * AUG), dtype=np.float32)
        xa_view = xa_full.reshape(NH, 128, KBLKS, AUG)
        xa_view[..., DH] = 1.0
        xa_view[..., 0:DH] = xh.reshape(KBLKS, 128, NH, DH).transpose(2, 1, 0, 3)
        ka = KBLKS * AUG
        xa_pair = np.ascontiguousarray(
            xa_full.reshape(NPAIR, 2, 128, ka).transpose(0, 2, 1, 3)
            .reshape(NPAIR, 128, 2 * ka)
        )
        xt_b = _to_bf16(xt_full)
        in_maps.append(
            {
                "xt": xt_b,
                "xq": np.ascontiguousarray(xt_b[:, :, 0:QB]),
                "xa": _to_bf16(xa_pair),
                "wt": wt_b,
                "bias": bias_b,
                "ones": ones_b,
                "ident": ident_b,
            }
        )
        meta.append((b, qlo))
    return in_maps, meta


_NC_CACHE = {}


def kernel(key, W_ctx, b_ctx):
    from concourse.bass_utils import run_bass_kernel_spmd

    key = np.asarray(key, dtype=np.float32)
    if "nc" not in _NC_CACHE:
        _NC_CACHE["nc"] = build_nc()
    nc = _NC_CACHE["nc"]
    in_maps, meta = shard_inputs(key, W_ctx, b_ctx)
    res = run_bass_kernel_spmd(nc, in_maps, list(range(NCORES)))
    outf = np.empty((B, L, H), dtype=np.float32)
    for c, (b, qlo) in enumerate(meta):
        outf[b, qlo : qlo + QB] = res.results[c]["out"]
    return outf


# revision 28
# speedup vs baseline: 1.0436x; 1.0436x over previous
"""Multi-head self-attention (QK^T -> softmax -> ctx -> linear) on 8 TRN2 cores.

Sharding: each core owns one (batch, query-block) shard: batch = core//4,
queries [qlo, qlo+512) with qlo = (core%4)*512. Attention needs all keys of
the core's batch, so keys are replicated per batch; no collectives needed.

Per core (head h, its 512 queries q, all 2048 keys k), all operands bf16:
  S_T[k, q]  = sum_d x[k, hd+d] * x[q, hd+d]            (PE, psum f32)
  P_T[k, q]  = exp(0.125 * S_T[k, q])                   (ACT, psum->sbuf bf16)
  ctx[q, m]  = sum_k P_T[k, q] * xa[k, m]               (PE; P parked as the
               stationary operand so only the 65-wide xa side streams;
               m = 64 dims + ones column -> softmax denominator at m=64)
  chunk[q,i] = ctx[q, d] / ctx[q, 64]                   (DVE tensor_scalar,
               per-partition recip scalar; -> bf16)
  chunkT     = transpose(chunk)                         (DMA xbar transpose)
  out[q, o]  = sum_i chunkT[i, q] * W[o, i] + b[o]      (PE + DVE adds)

The PE cost model charges only streamed output columns (stationary loads are
free), so parking P halves ctx cost vs streaming it; everything else is
orientation-chosen to keep output partitions full.
"""

import sys

for _p in ("/opt/trn_rl_repo", "/root/.axon_site/_ro/trn_rl_repo"):
    if _p not in sys.path:
        sys.path.append(_p)

import numpy as np

import concourse.bacc as bacc
import concourse.bass as bass
import concourse.mybir as mybir
import concourse.tile as tile

F32 = mybir.dt.float32
BF16 = mybir.dt.bfloat16

B, L, H, NH, DH = 2, 2048, 1024, 16, 64
NCORES = 8
QB = 512
KBLKS = L // 128          # 16 key blocks
NPAIR = NH // 2           # 8 head pairs
AUG = DH + 1              # 64 dims + ones column
HC = H // 128             # 8 hidden chunks (one per pair)
OBW = 512                 # proj output column block
SCALE = float(1.0 / np.sqrt(DH))
QSUB = QB // 128          # 4 query subtiles


def build_nc():
    nc = bacc.Bacc("TRN2")
    xt = nc.declare_dram_parameter("xt", [NPAIR, 128, L], BF16, isOutput=False)
    xq = nc.declare_dram_parameter("xq", [NPAIR, 128, QB], BF16, isOutput=False)
    xa = nc.declare_dram_parameter("xa", [NPAIR, 128, 2 * KBLKS * AUG], BF16, isOutput=False)
    wt = nc.declare_dram_parameter("wt", [128, HC * H], BF16, isOutput=False)
    bias = nc.declare_dram_parameter("bias", [1, H], BF16, isOutput=False)
    ones = nc.declare_dram_parameter("ones", [1, 128], BF16, isOutput=False)
    ident = nc.declare_dram_parameter("ident", [128, 128], BF16, isOutput=False)
    out = nc.declare_dram_parameter("out", [QB, H], F32, isOutput=True)

    NSTEP = NPAIR * KBLKS
    SKEW = 2
    # scheduling floor per global step (ns): keeps the tile scheduler from
    # hoisting proj work into much earlier PE positions, where an unmet
    # transpose dep would stall the PE counter that gates the exp stream.
    TSTEP_NS = 1040.0
    TBASE_NS = 4600.0

    def floor_ms(gs):
        return (TBASE_NS + gs * TSTEP_NS) / 1e6

    # Schraudolph-pair constants (bf16 domain): t = scores*SCALE*log2(e);
    # int16 = t*128 + 127*128 + C, with C folding the /2 normalization
    EXPA = float(128.0 * SCALE * 1.4426950408889634)
    # +22.375: scale calibration so the equal-weight pair averages to 1.0x
    # of true exp (matching the ACT-exp key blocks in the same softmax)
    EXPB1 = float(127 * 128 - 128 + 22.375)
    EXPB2 = float(127 * 128 - 192 + 22.375)
    # offload in runs of 3 so the psum-slot refill chain is amortized once
    # per run instead of once per step
    OFF_STARTS = globals().get('_OFF_STARTS', ())
    OFFLOAD = frozenset(
        g for start in OFF_STARTS for g in (start, start + 1, start + 2)
    )
    OFF_SKEW = 5

    with tile.TileContext(nc) as tc:
        with (
            tc.tile_pool(name="xt", bufs=2) as xt_pool,
            tc.tile_pool(name="xq", bufs=2) as xq_pool,
            tc.tile_pool(name="xa", bufs=2) as xa_pool,
            tc.tile_pool(name="p", bufs=10) as p_pool,
            tc.tile_pool(name="cq", bufs=8) as cq_pool,
            tc.tile_pool(name="rc", bufs=4) as rc_pool,
            tc.tile_pool(name="y", bufs=4) as y_pool,
            tc.tile_pool(name="consts", bufs=1) as consts,
            tc.tile_pool(name="spsum", bufs=2, space="PSUM") as s_psum,
            tc.tile_pool(name="ctxpsum", bufs=3, space="PSUM") as ctx_psum,
            tc.tile_pool(name="prjpsum", bufs=1, space="PSUM") as prj_psum,
        ):
            ones_t = consts.tile([1, 128], BF16)
            ident_t = consts.tile([128, 128], BF16)
            bias_t = consts.tile([1, H], BF16)
            bias_bc = consts.tile([128, H], F32)

            wt_ts = [
                consts.tile([128, H], BF16, tag=f"wt{c}", name=f"wt{c}")
                for c in range(HC)
            ]
            chunks = [
                consts.tile([128, QB], BF16, tag=f"ch{c}", name=f"ch{c}")
                for c in range(HC)
            ]
            acc = [
                consts.tile([128, H], F32, tag=f"acc{q}", name=f"acc{q}")
                for q in range(QSUB)
            ]

            def pair_dmas(pr, split_xt=False):
                xt_t = xt_pool.tile([128, L], BF16, tag="xt", name=f"xt{pr}")
                xq_t = xq_pool.tile([128, QB], BF16, tag="xq", name=f"xq{pr}")
                nc.sync.dma_start(xq_t[:], xq[pr])
                if split_xt:
                    # first key quarter lands fast so scores can start early
                    nc.sync.dma_start(xt_t[:, 0:512], xt[pr][:, 0:512])
                    nc.sync.dma_start(xt_t[:, 512:L], xt[pr][:, 512:L])
                else:
                    nc.sync.dma_start(xt_t[:], xt[pr])
                xa_t = xa_pool.tile([128, 2 * KBLKS * AUG], BF16, tag="xa", name=f"xa{pr}")
                nc.sync.dma_start(xa_t[:], xa[pr])
                return [xt_t, xq_t, xa_t, None, None]

            def emit_proj_group(clist, qs, ob, pool, tag, eng=None):
                obsl = slice(ob * OBW, (ob + 1) * OBW)
                qsl = slice(qs * 128, (qs + 1) * 128)
                cp = pool.tile([128, 2 * QB] if tag == "s" else [128, OBW],
                               F32, tag=tag, name=f"cp{clist[0]}_{qs}_{ob}")
                for i, c in enumerate(clist):
                    nc.tensor.matmul(
                        cp[:, 0:OBW], chunks[c][:, qsl], wt_ts[c][:, obsl],
                        start=(i == 0), stop=(i == len(clist) - 1),
                    )
                prev = bias_bc if clist[0] == 0 else acc[qs]
                (eng or nc.vector).tensor_add(
                    acc[qs][:, obsl], cp[:, 0:OBW], prev[:, obsl]
                )

            def emit_drain(prp, ctx_a, ctx_b):
                # normalize pair prp's ctx accumulators and transpose into
                # the proj chunk tile; for the final pair, pipeline the
                # remaining projection groups and output DMAs per q-subtile
                last = prp == NPAIR - 1
                rc_a = rc_pool.tile([128, QSUB], F32, tag="rc", name=f"rca{prp}")
                rc_b = rc_pool.tile([128, QSUB], F32, tag="rc", name=f"rcb{prp}")
                with nc.allow_low_precision(reason="softmax denominator recip"):
                    nc.vector.reciprocal(rc_a[:], ctx_a[:, DH : QSUB * AUG : AUG])
                    nc.vector.reciprocal(rc_b[:], ctx_b[:, DH : QSUB * AUG : AUG])
                for qs in range(QSUB):
                    cq = cq_pool.tile([128, 128], BF16, tag="cq", name=f"cq{prp}_{qs}")
                    nc.vector.tensor_scalar_mul(
                        cq[:, 0:DH], ctx_a[:, qs * AUG : qs * AUG + DH],
                        rc_a[:, qs : qs + 1],
                    )
                    nc.vector.tensor_scalar_mul(
                        cq[:, DH : 2 * DH], ctx_b[:, qs * AUG : qs * AUG + DH],
                        rc_b[:, qs : qs + 1],
                    )
                    if not last:
                        nc.sync.dma_start_transpose(
                            chunks[prp][:, qs * 128 : (qs + 1) * 128], cq[:, :]
                        )
                        continue
                    # final pair: PE-transpose (HWDGE is slow to drain) + ACT
                    # copy; fused (5,6,7) groups follow in a second pass so
                    # the DVE adds overlap later q-subtiles' normalizes
                    tp = s_psum.tile([128, 128], BF16, tag="s", name=f"tp{qs}")
                    nc.tensor.transpose(tp[:], cq[:, :], ident_t[:])
                    qsl = slice(qs * 128, (qs + 1) * 128)
                    nc.scalar.copy(chunks[prp][:, qsl], tp[:])
                if last:
                    for qs in range(QSUB):
                        qsl = slice(qs * 128, (qs + 1) * 128)
                        clist = (5, 6, 7) if qs == 0 else (7,)
                        cp = s_psum.tile([128, 2 * QB], F32, tag="s", name=f"tcp{qs}")
                        for ob in range(2):
                            obsl = slice(ob * OBW, (ob + 1) * OBW)
                            for i, c in enumerate(clist):
                                nc.tensor.matmul(
                                    cp[:, obsl], chunks[c][:, qsl], wt_ts[c][:, obsl],
                                    start=(i == 0), stop=(i == len(clist) - 1),
                                )
                        if qs < QSUB - 1:
                            nc.vector.tensor_add(acc[qs][:], cp[:], acc[qs][:])
                            nc.sync.dma_start(out[qsl, :], acc[qs][:])
                        else:
                            # quarter-split so the final DMA transfer is small
                            for q4 in range(4):
                                sl = slice(q4 * 256, (q4 + 1) * 256)
                                nc.vector.tensor_add(
                                    acc[qs][:, sl], cp[:, sl], acc[qs][:, sl]
                                )
                                nc.sync.dma_start(out[qsl, sl], acc[qs][:, sl])

            # proj emission plan: step -> (chunk list, qs, ob)
            # chunk-pairs (0,1)@pairs2-3, (2,3)@pairs4-5; chunk 4 singles@pair6,
            # (5,6)@pair7, chunk 7 + leftovers in the tail.
            # window steps start ~4 kbs after the newer chunk's drain so the
            # first group never waits on an in-flight transpose
            plan = {}
            for hp, base_pr in ((0, 2), (1, 4)):
                cl = [2 * hp, 2 * hp + 1]
                steps = [base_pr * KBLKS + k for k in (6, 8, 10, 12, 14)] + [
                    (base_pr + 1) * KBLKS + k for k in (4, 8, 12)
                ]
                for gidx, st in enumerate(steps):
                    qs, ob = divmod(gidx, 2)
                    plan[st] = (cl, qs, ob)
            for i, kbw in enumerate((3, 5, 7, 9, 11, 13, 15)):
                plan[6 * KBLKS + kbw] = ([4], i // 2, i % 2)
            plan[7 * KBLKS + 1] = ([4], 3, 1)
            for i, kbw in enumerate((5, 7, 9, 11, 13, 15)):
                plan[7 * KBLKS + kbw] = ([5, 6], 1 + i // 2, i % 2)

            tiles = {}
            pipe = []
            y1s = {}
            diag_p = {}
            for gs in range(NSTEP + OFF_SKEW):
                cur = None
                if gs < NSTEP:
                    pr, kb = divmod(gs, KBLKS)
                    if pr == 0 and kb == 0:
                        warm = consts.tile([1, 512], BF16)
                        nc.vector.memset(warm[:], 1.0)
                        for wi in range(8):
                            wps = prj_psum.tile([128, OBW], F32, tag="prj",
                                                name=f"warm{wi}")
                            nc.tensor.matmul(
                                wps[:], warm[0:1, 0:128], warm[0:1, :],
                                start=True, stop=True,
                            )
                        tiles[0] = pair_dmas(0, split_xt=True)
                        nc.sync.dma_start(wt_ts[0][:], wt[:, 0:H])
                        nc.sync.dma_start(ones_t[:], ones[:])
                        nc.sync.dma_start(bias_t[:], bias[:])
                        nc.sync.dma_start(ident_t[:], ident[:])
                    if pr == 0 and kb == 6:
                        # bias broadcast across partitions via K=1 ones matmul
                        # (uses the proj psum bank, idle until pair 2, so the
                        # late-landing bias DMA never stalls the score slots)
                        for ob in range(H // OBW):
                            obsl = slice(ob * OBW, (ob + 1) * OBW)
                            bps = prj_psum.tile([128, OBW], F32, tag="prj")
                            nc.tensor.matmul(
                                bps[:], ones_t[0:1, :], bias_t[0:1, obsl],
                                start=True, stop=True,
                            )
                            nc.vector.tensor_copy(bias_bc[:, obsl], bps[:])
                    if kb == 2 and pr + 1 < NPAIR:
                        # floor keeps prefetches behind the previous
                        # drain's transposes in the HWDGE queue
                        with tc.tile_wait_until(floor_ms((pr + 1) * KBLKS - 6)):
                            tiles[pr + 1] = pair_dmas(pr + 1)
                    if kb == 4 and pr + 1 < NPAIR:
                        with tc.tile_wait_until(floor_ms(min((pr + 3) * KBLKS - 10, NSTEP))):
                            nc.sync.dma_start(
                                wt_ts[pr + 1][:], wt[:, (pr + 1) * H : (pr + 2) * H]
                            )
                    xt_t, xq_t, xa_t, _, _ = tiles[pr]
                    ksl = slice(kb * 128, (kb + 1) * 128)
                    # keys are permuted per core so kb 12-15 is the core's
                    # own query block: S(i,j) = S(j,i)^T there, so compute
                    # only q-subtiles b <= kb-12 and fill the rest by
                    # transposing later steps' exp'd blocks (exact, on the
                    # otherwise idle DMA engines)
                    dw = (kb - 11) * 128 if kb >= KBLKS - QSUB else QB
                    s_ab = s_psum.tile([128, 2 * QB], F32, tag="s")
                    nc.tensor.matmul(
                        s_ab[:, 0:dw], xt_t[0:64, ksl], xq_t[0:64, 0:dw],
                        start=True, stop=True,
                    )
                    nc.tensor.matmul(
                        s_ab[:, QB : QB + dw], xt_t[64:128, ksl],
                        xq_t[64:128, 0:dw],
                        start=True, stop=True,
                    )
                    if gs in OFFLOAD:
                        # bit-trick exp, step 1 of 2: y1 = int16(s*A + B1).
                        # This is the only psum reader, so the score slot is
                        # released at ACT-parity latency; the combine runs two
                        # steps later (see the finish block below).
                        y1 = y_pool.tile([128, 2 * QB], mybir.dt.int16,
                                         tag="y1", name=f"y1_{gs}")
                        nc.vector.tensor_scalar(
                            y1[:], s_ab[:], EXPA, EXPB1,
                            op0=mybir.AluOpType.mult, op1=mybir.AluOpType.add,
                        )
                        y1s[gs] = y1
                        cur = (gs, None)
                    else:
                        p_ab = p_pool.tile([128, 2 * QB], BF16, tag="p")
                        if kb >= KBLKS - QSUB:
                            a = kb - (KBLKS - QSUB)
                            sv = s_ab.rearrange("p (h q) -> p h q", h=2)
                            pv = p_ab.rearrange("p (h q) -> p h q", h=2)
                            nc.scalar.activation(
                                pv[:, :, 0:dw], sv[:, :, 0:dw],
                                mybir.ActivationFunctionType.Exp, scale=SCALE,
                            )
                            diag_p[a] = p_ab
                            for b in range(a):
                                for hh in range(2):
                                    nc.sync.dma_start_transpose(
                                        diag_p[b][:, hh * QB + a * 128
                                                  : hh * QB + (a + 1) * 128],
                                        p_ab[:, hh * QB + b * 128
                                             : hh * QB + (b + 1) * 128],
                                    )
                        else:
                            nc.scalar.activation(
                                p_ab[:], s_ab[:],
                                mybir.ActivationFunctionType.Exp, scale=SCALE,
                            )
                        cur = (gs, p_ab)
                if cur is not None:
                    pipe.append(cur)
                tfin = gs - 2
                if tfin in OFFLOAD:
                    # bit-trick exp, step 2: P = S(B1) + S(B1-64); the pair
                    # cancels the interpolation ripple to ~0.6% rms and
                    # round(t-64) == round(t)-64 exactly -> int16 subtract
                    y1 = y1s.pop(tfin)
                    y2 = y_pool.tile([128, 2 * QB], mybir.dt.int16,
                                     tag="y2", name=f"y2_{tfin}")
                    nc.vector.tensor_scalar_sub(y2[:], y1[:], 64)
                    p_fin = p_pool.tile([128, 2 * QB], BF16, tag="p",
                                        name=f"pf{tfin}")
                    nc.vector.tensor_add(
                        p_fin[:], y2[:].bitcast(BF16), y1[:].bitcast(BF16)
                    )
                    for pi, (pgs, pv) in enumerate(pipe):
                        if pgs == tfin:
                            pipe[pi] = (pgs, p_fin)
                            break
                prevs = []
                while pipe and len(prevs) < 3:
                    hkb = pipe[0][0] % KBLKS
                    need = OFF_SKEW if pipe[0][0] in OFFLOAD else (
                        max(SKEW, 18 - hkb) if hkb >= KBLKS - QSUB else SKEW
                    )
                    if gs - pipe[0][0] >= need or gs >= NSTEP + SKEW - 1:
                        prevs.append(pipe.pop(0))
                    else:
                        break
                for prev in prevs:
                    gsp, pp = prev
                    prp, kbp = divmod(gsp, KBLKS)
                    if kbp == 0:
                        ctx_a = ctx_psum.tile([128, QSUB * AUG], F32, tag="ctx",
                                              name=f"ctxa{prp}")
                        ctx_b = ctx_psum.tile([128, QSUB * AUG], F32, tag="ctx",
                                              name=f"ctxb{prp}")
                        tiles[prp][3] = ctx_a
                        tiles[prp][4] = ctx_b
                    _, _, xa_t, ctx_a, ctx_b = tiles[prp]
                    for j, ctx_t in ((0, ctx_a), (1, ctx_b)):
                        asl = slice(j * KBLKS * AUG + kbp * AUG,
                                    j * KBLKS * AUG + (kbp + 1) * AUG)
                        for qs in range(QSUB):
                            # The 4 qs accumulation groups share one psum
                            # bank (= one 2KB zero region): only the tile's
                            # first matmul may set start (start marks the
                            # WHOLE region pending-zero, so later groups'
                            # first writes still overwrite-not-accumulate),
                            # and only the last sets stop.
                            nc.tensor.matmul(
                                ctx_t[:, qs * AUG : (qs + 1) * AUG],
                                pp[:, j * QB + qs * 128 : j * QB + (qs + 1) * 128],
                                xa_t[:, asl],
                                start=(kbp == 0 and qs == 0),
                                stop=(kbp == KBLKS - 1 and qs == QSUB - 1),
                            )
                    g = plan.get(gsp)
                    if g is not None:
                        with tc.tile_wait_until(floor_ms(gsp)):
                            emit_proj_group(g[0], g[1], g[2], prj_psum, "prj")
                    if kbp == KBLKS - 1:
                        emit_drain(prp, ctx_a, ctx_b)
    nc.compile()
    return nc


def _to_bf16(x):
    import ml_dtypes

    return np.asarray(x, np.float32).astype(ml_dtypes.bfloat16)


def shard_inputs(key, W_ctx, b_ctx):
    """Host-side prep of per-core input dicts (bf16 layouts).

    Keys are permuted per core so the core's own query block comes first:
    softmax is key-order invariant, and this puts the symmetric diagonal
    score block at kb 0..3 identically on every core (SPMD-friendly)."""
    Bv = key.shape[0]
    cores_per_batch = NCORES // Bv

    key = np.asarray(key, dtype=np.float32)
    wt_host = np.ascontiguousarray(
        np.asarray(W_ctx, np.float32).T.reshape(HC, 128, H).transpose(1, 0, 2)
        .reshape(128, HC * H)
    )
    bias_host = np.asarray(b_ctx, np.float32).reshape(1, H)
    wt_b = _to_bf16(wt_host)
    bias_b = _to_bf16(bias_host)
    ones_b = _to_bf16(np.ones((1, 128), dtype=np.float32))
    ident_b = _to_bf16(np.eye(128, dtype=np.float32))

    in_maps = []
    meta = []
    for c in range(NCORES):
        b = c // cores_per_batch
        qlo = (c % cores_per_batch) * QB
        xp = np.concatenate(
            [key[b, :qlo], key[b, qlo + QB :], key[b, qlo : qlo + QB]], axis=0
        )
        xh = xp.reshape(L, NH, DH)
        # xt: [NPAIR, 128, L]; pair p rows 0:64 = head 2p, 64:128 = head 2p+1
        xt_full = np.ascontiguousarray(
            xh.transpose(1, 2, 0).reshape(NPAIR, 2 * DH, L)
        )
        # xa: [NH, 128, KBLKS*AUG] with ones at kb*AUG+DH, then pair-merged
        xa_full = np.empty((NH, 128, KBLKS * AUG), dtype=np.float32)
        xa_view = xa_full.reshape(NH, 128, KBLKS, AUG)
        xa_view[..., DH] = 1.0
        xa_view[..., 0:DH] = xh.reshape(KBLKS, 128, NH, DH).transpose(2, 1, 0, 3)
        ka = KBLKS * AUG
        xa_pair = np.ascontiguousarray(
            xa_full.reshape(NPAIR, 2, 128, ka).transpose(0, 2, 1, 3)
            .reshape(NPAIR, 128, 2 * ka)
        )
        xt_b = _to_bf16(xt_full)
        in_maps.append(
            {
                "xt": xt_b,
                "xq": np.ascontiguousarray(xt_b[:, :, L - QB : L]),
                "xa": _to_bf16(xa_pair),
                "wt": wt_b,
                "bias": bias_b,
                "ones": ones_b,
                "ident": ident_b,
            }
        )
        meta.append((b, qlo))
    return in_maps, meta


_NC_CACHE = {}


def kernel(key, W_ctx, b_ctx):
    from concourse.bass_utils import run_bass_kernel_spmd

    key = np.asarray(key, dtype=np.float32)
    if "nc" not in _NC_CACHE:
        _NC_CACHE["nc"] = build_nc()
    nc = _NC_CACHE["nc"]
    in_maps, meta = shard_inputs(key, W_ctx, b_ctx)
    res = run_bass_kernel_spmd(nc, in_maps, list(range(NCORES)))
    outf = np.empty((B, L, H), dtype=np.float32)
    for c, (b, qlo) in enumerate(meta):
        outf[b, qlo : qlo + QB] = res.results[c]["out"]
    return outf


# revision 29
# speedup vs baseline: 1.3334x; 1.2777x over previous
"""Multi-head self-attention (QK^T -> softmax -> ctx -> linear) on 8 TRN2 cores.

Sharding: each core owns one (batch, query-block) shard: batch = core//4,
queries [qlo, qlo+512) with qlo = (core%4)*512. Attention needs all keys of
the core's batch, so keys are replicated per batch; no collectives needed.

Per core (head h, its 512 queries q, all 2048 keys k), all operands bf16:
  S_T[k, q]  = sum_d x[k, hd+d] * x[q, hd+d]            (PE, psum f32)
  P_T[k, q]  = exp(0.125 * S_T[k, q])                   (ACT, psum->sbuf bf16)
  ctx[q, m]  = sum_k P_T[k, q] * xa[k, m]               (PE; P parked as the
               stationary operand so only the 65-wide xa side streams;
               m = 64 dims + ones column -> softmax denominator at m=64)
  chunk[q,i] = ctx[q, d] / ctx[q, 64]                   (DVE tensor_scalar,
               per-partition recip scalar; -> bf16)
  chunkT     = transpose(chunk)                         (DMA xbar transpose)
  out[q, o]  = sum_i chunkT[i, q] * W[o, i] + b[o]      (PE + DVE adds)

The PE cost model charges only streamed output columns (stationary loads are
free), so parking P halves ctx cost vs streaming it; everything else is
orientation-chosen to keep output partitions full.
"""

import sys

for _p in ("/opt/trn_rl_repo", "/root/.axon_site/_ro/trn_rl_repo"):
    if _p not in sys.path:
        sys.path.append(_p)

import numpy as np

import concourse.bacc as bacc
import concourse.bass as bass
import concourse.mybir as mybir
import concourse.tile as tile

F32 = mybir.dt.float32
BF16 = mybir.dt.bfloat16

B, L, H, NH, DH = 2, 2048, 1024, 16, 64
NCORES = 8
QB = 512
KBLKS = L // 128          # 16 key blocks
NPAIR = NH // 2           # 8 head pairs
AUG = DH + 1              # 64 dims + ones column
HC = H // 128             # 8 hidden chunks (one per pair)
OBW = 512                 # proj output column block
SCALE = float(1.0 / np.sqrt(DH))
QSUB = QB // 128          # 4 query subtiles


def build_nc():
    nc = bacc.Bacc("TRN2")
    xt = nc.declare_dram_parameter("xt", [NPAIR, 128, L], BF16, isOutput=False)
    xq = nc.declare_dram_parameter("xq", [NPAIR, 128, QB], BF16, isOutput=False)
    xa = nc.declare_dram_parameter("xa", [NPAIR, 128, 2 * KBLKS * AUG], BF16, isOutput=False)
    wt = nc.declare_dram_parameter("wt", [128, HC * H], BF16, isOutput=False)
    bias = nc.declare_dram_parameter("bias", [1, H], BF16, isOutput=False)
    ones = nc.declare_dram_parameter("ones", [1, 128], BF16, isOutput=False)
    ident = nc.declare_dram_parameter("ident", [128, 128], BF16, isOutput=False)
    out = nc.declare_dram_parameter("out", [QB, H], F32, isOutput=True)

    NSTEP = NPAIR * KBLKS
    SKEW = 2
    # scheduling floor per global step (ns): keeps the tile scheduler from
    # hoisting proj work into much earlier PE positions, where an unmet
    # transpose dep would stall the PE counter that gates the exp stream.
    TSTEP_NS = 1040.0
    TBASE_NS = 4600.0

    def floor_ms(gs):
        return (TBASE_NS + gs * TSTEP_NS) / 1e6

    # Schraudolph-pair constants (bf16 domain): t = scores*SCALE*log2(e);
    # int16 = t*128 + 127*128 + C, with C folding the /2 normalization
    EXPA = float(128.0 * SCALE * 1.4426950408889634)
    # +22.375: scale calibration so the equal-weight pair averages to 1.0x
    # of true exp (matching the ACT-exp key blocks in the same softmax)
    EXPB1 = float(127 * 128 - 128 + 22.375)
    EXPB2 = float(127 * 128 - 192 + 22.375)
    # offload in runs of 3 so the psum-slot refill chain is amortized once
    # per run instead of once per step
    OFF_STARTS = globals().get('_OFF_STARTS', ())
    DIAG_ON = bool(globals().get('_DIAG_ON', False))
    OFFLOAD = frozenset(
        g for start in OFF_STARTS for g in (start, start + 1, start + 2)
    )
    OFF_SKEW = 5

    with tile.TileContext(nc) as tc:
        with (
            tc.tile_pool(name="xt", bufs=2) as xt_pool,
            tc.tile_pool(name="xq", bufs=2) as xq_pool,
            tc.tile_pool(name="xa", bufs=2) as xa_pool,
            tc.tile_pool(name="p", bufs=10) as p_pool,
            tc.tile_pool(name="cq", bufs=8) as cq_pool,
            tc.tile_pool(name="rc", bufs=4) as rc_pool,
            tc.tile_pool(name="y", bufs=4) as y_pool,
            tc.tile_pool(name="consts", bufs=1) as consts,
            tc.tile_pool(name="spsum", bufs=2, space="PSUM") as s_psum,
            tc.tile_pool(name="ctxpsum", bufs=3, space="PSUM") as ctx_psum,
            tc.tile_pool(name="prjpsum", bufs=1, space="PSUM") as prj_psum,
        ):
            ones_t = consts.tile([1, 128], BF16)
            ident_t = consts.tile([128, 128], BF16)
            bias_t = consts.tile([1, H], BF16)
            bias_bc = consts.tile([128, H], F32)

            wt_ts = [
                consts.tile([128, H], BF16, tag=f"wt{c}", name=f"wt{c}")
                for c in range(HC)
            ]
            chunks = [
                consts.tile([128, QB], BF16, tag=f"ch{c}", name=f"ch{c}")
                for c in range(HC)
            ]
            acc = [
                consts.tile([128, H], F32, tag=f"acc{q}", name=f"acc{q}")
                for q in range(QSUB)
            ]

            def pair_dmas(pr, split_xt=False):
                xt_t = xt_pool.tile([128, L], BF16, tag="xt", name=f"xt{pr}")
                xq_t = xq_pool.tile([128, QB], BF16, tag="xq", name=f"xq{pr}")
                nc.sync.dma_start(xq_t[:], xq[pr])
                if split_xt:
                    # first key quarter lands fast so scores can start early
                    nc.sync.dma_start(xt_t[:, 0:512], xt[pr][:, 0:512])
                    nc.sync.dma_start(xt_t[:, 512:L], xt[pr][:, 512:L])
                else:
                    nc.sync.dma_start(xt_t[:], xt[pr])
                xa_t = xa_pool.tile([128, 2 * KBLKS * AUG], BF16, tag="xa", name=f"xa{pr}")
                nc.sync.dma_start(xa_t[:], xa[pr])
                return [xt_t, xq_t, xa_t, None, None]

            def emit_proj_group(clist, qs, ob, pool, tag, eng=None):
                obsl = slice(ob * OBW, (ob + 1) * OBW)
                qsl = slice(qs * 128, (qs + 1) * 128)
                cp = pool.tile([128, 2 * QB] if tag == "s" else [128, OBW],
                               F32, tag=tag, name=f"cp{clist[0]}_{qs}_{ob}")
                for i, c in enumerate(clist):
                    nc.tensor.matmul(
                        cp[:, 0:OBW], chunks[c][:, qsl], wt_ts[c][:, obsl],
                        start=(i == 0), stop=(i == len(clist) - 1),
                    )
                prev = bias_bc if clist[0] == 0 else acc[qs]
                (eng or nc.vector).tensor_add(
                    acc[qs][:, obsl], cp[:, 0:OBW], prev[:, obsl]
                )

            def emit_drain(prp, ctx_a, ctx_b):
                # normalize pair prp's ctx accumulators and transpose into
                # the proj chunk tile; for the final pair, pipeline the
                # remaining projection groups and output DMAs per q-subtile
                last = prp == NPAIR - 1
                rc_a = rc_pool.tile([128, QSUB], F32, tag="rc", name=f"rca{prp}")
                rc_b = rc_pool.tile([128, QSUB], F32, tag="rc", name=f"rcb{prp}")
                with nc.allow_low_precision(reason="softmax denominator recip"):
                    nc.vector.reciprocal(rc_a[:], ctx_a[:, DH : QSUB * AUG : AUG])
                    nc.vector.reciprocal(rc_b[:], ctx_b[:, DH : QSUB * AUG : AUG])
                for qs in range(QSUB):
                    cq = cq_pool.tile([128, 128], BF16, tag="cq", name=f"cq{prp}_{qs}")
                    nc.vector.tensor_scalar_mul(
                        cq[:, 0:DH], ctx_a[:, qs * AUG : qs * AUG + DH],
                        rc_a[:, qs : qs + 1],
                    )
                    nc.vector.tensor_scalar_mul(
                        cq[:, DH : 2 * DH], ctx_b[:, qs * AUG : qs * AUG + DH],
                        rc_b[:, qs : qs + 1],
                    )
                    if not last:
                        nc.sync.dma_start_transpose(
                            chunks[prp][:, qs * 128 : (qs + 1) * 128], cq[:, :]
                        )
                        continue
                    # final pair: PE-transpose (HWDGE is slow to drain) + ACT
                    # copy; fused (5,6,7) groups follow in a second pass so
                    # the DVE adds overlap later q-subtiles' normalizes
                    tp = s_psum.tile([128, 128], BF16, tag="s", name=f"tp{qs}")
                    nc.tensor.transpose(tp[:], cq[:, :], ident_t[:])
                    qsl = slice(qs * 128, (qs + 1) * 128)
                    nc.scalar.copy(chunks[prp][:, qsl], tp[:])
                if last:
                    for qs in range(QSUB):
                        qsl = slice(qs * 128, (qs + 1) * 128)
                        clist = (5, 6, 7) if qs == 0 else (7,)
                        cp = s_psum.tile([128, 2 * QB], F32, tag="s", name=f"tcp{qs}")
                        for ob in range(2):
                            obsl = slice(ob * OBW, (ob + 1) * OBW)
                            for i, c in enumerate(clist):
                                nc.tensor.matmul(
                                    cp[:, obsl], chunks[c][:, qsl], wt_ts[c][:, obsl],
                                    start=(i == 0), stop=(i == len(clist) - 1),
                                )
                        if qs < QSUB - 1:
                            nc.vector.tensor_add(acc[qs][:], cp[:], acc[qs][:])
                            nc.sync.dma_start(out[qsl, :], acc[qs][:])
                        else:
                            # quarter-split so the final DMA transfer is small
                            for q4 in range(4):
                                sl = slice(q4 * 256, (q4 + 1) * 256)
                                nc.vector.tensor_add(
                                    acc[qs][:, sl], cp[:, sl], acc[qs][:, sl]
                                )
                                nc.sync.dma_start(out[qsl, sl], acc[qs][:, sl])

            # proj emission plan: step -> (chunk list, qs, ob)
            # chunk-pairs (0,1)@pairs2-3, (2,3)@pairs4-5; chunk 4 singles@pair6,
            # (5,6)@pair7, chunk 7 + leftovers in the tail.
            # window steps start ~4 kbs after the newer chunk's drain so the
            # first group never waits on an in-flight transpose
            plan = {}
            for hp, base_pr in ((0, 2), (1, 4)):
                cl = [2 * hp, 2 * hp + 1]
                steps = [base_pr * KBLKS + k for k in (6, 8, 10, 12, 14)] + [
                    (base_pr + 1) * KBLKS + k for k in (4, 8, 12)
                ]
                for gidx, st in enumerate(steps):
                    qs, ob = divmod(gidx, 2)
                    plan[st] = (cl, qs, ob)
            for i, kbw in enumerate((3, 5, 7, 9, 11, 13, 15)):
                plan[6 * KBLKS + kbw] = ([4], i // 2, i % 2)
            plan[7 * KBLKS + 1] = ([4], 3, 1)
            for i, kbw in enumerate((5, 7, 9, 11, 13, 15)):
                plan[7 * KBLKS + kbw] = ([5, 6], 1 + i // 2, i % 2)

            tiles = {}
            pipe = []
            y1s = {}
            diag_p = {}
            for gs in range(NSTEP + OFF_SKEW):
                cur = None
                if gs < NSTEP:
                    pr, kb = divmod(gs, KBLKS)
                    if pr == 0 and kb == 0:
                        warm = consts.tile([1, 512], BF16)
                        nc.vector.memset(warm[:], 1.0)
                        for wi in range(8):
                            wps = prj_psum.tile([128, OBW], F32, tag="prj",
                                                name=f"warm{wi}")
                            nc.tensor.matmul(
                                wps[:], warm[0:1, 0:128], warm[0:1, :],
                                start=True, stop=True,
                            )
                        tiles[0] = pair_dmas(0, split_xt=True)
                        nc.sync.dma_start(wt_ts[0][:], wt[:, 0:H])
                        nc.sync.dma_start(ones_t[:], ones[:])
                        nc.sync.dma_start(bias_t[:], bias[:])
                        nc.sync.dma_start(ident_t[:], ident[:])
                    if pr == 0 and kb == 6:
                        # bias broadcast across partitions via K=1 ones matmul
                        # (uses the proj psum bank, idle until pair 2, so the
                        # late-landing bias DMA never stalls the score slots)
                        for ob in range(H // OBW):
                            obsl = slice(ob * OBW, (ob + 1) * OBW)
                            bps = prj_psum.tile([128, OBW], F32, tag="prj")
                            nc.tensor.matmul(
                                bps[:], ones_t[0:1, :], bias_t[0:1, obsl],
                                start=True, stop=True,
                            )
                            nc.vector.tensor_copy(bias_bc[:, obsl], bps[:])
                    if kb == 2 and pr + 1 < NPAIR:
                        # floor keeps prefetches behind the previous
                        # drain's transposes in the HWDGE queue
                        with tc.tile_wait_until(floor_ms((pr + 1) * KBLKS - 6)):
                            tiles[pr + 1] = pair_dmas(pr + 1)
                    if kb == 4 and pr + 1 < NPAIR:
                        with tc.tile_wait_until(floor_ms(min((pr + 3) * KBLKS - 10, NSTEP))):
                            nc.sync.dma_start(
                                wt_ts[pr + 1][:], wt[:, (pr + 1) * H : (pr + 2) * H]
                            )
                    xt_t, xq_t, xa_t, _, _ = tiles[pr]
                    ksl = slice(kb * 128, (kb + 1) * 128)
                    # keys are permuted per core so kb 12-15 is the core's
                    # own query block: S(i,j) = S(j,i)^T there, so compute
                    # only q-subtiles b <= kb-12 and fill the rest by
                    # transposing later steps' exp'd blocks (exact, on the
                    # otherwise idle DMA engines)
                    dw = (kb - 11) * 128 if (DIAG_ON and kb >= KBLKS - QSUB) else QB
                    s_ab = s_psum.tile([128, 2 * QB], F32, tag="s")
                    nc.tensor.matmul(
                        s_ab[:, 0:dw], xt_t[0:64, ksl], xq_t[0:64, 0:dw],
                        start=True, stop=True,
                    )
                    nc.tensor.matmul(
                        s_ab[:, QB : QB + dw], xt_t[64:128, ksl],
                        xq_t[64:128, 0:dw],
                        start=True, stop=True,
                    )
                    if gs in OFFLOAD:
                        # bit-trick exp, step 1 of 2: y1 = int16(s*A + B1).
                        # This is the only psum reader, so the score slot is
                        # released at ACT-parity latency; the combine runs two
                        # steps later (see the finish block below).
                        y1 = y_pool.tile([128, 2 * QB], mybir.dt.int16,
                                         tag="y1", name=f"y1_{gs}")
                        nc.vector.tensor_scalar(
                            y1[:], s_ab[:], EXPA, EXPB1,
                            op0=mybir.AluOpType.mult, op1=mybir.AluOpType.add,
                        )
                        y1s[gs] = y1
                        cur = (gs, None)
                    else:
                        p_ab = p_pool.tile([128, 2 * QB], BF16, tag="p")
                        if DIAG_ON and kb >= KBLKS - QSUB:
                            a = kb - (KBLKS - QSUB)
                            sv = s_ab.rearrange("p (h q) -> p h q", h=2)
                            pv = p_ab.rearrange("p (h q) -> p h q", h=2)
                            nc.scalar.activation(
                                pv[:, :, 0:dw], sv[:, :, 0:dw],
                                mybir.ActivationFunctionType.Exp, scale=SCALE,
                            )
                            diag_p[a] = p_ab
                            for b in range(a):
                                for hh in range(2):
                                    nc.sync.dma_start_transpose(
                                        diag_p[b][:, hh * QB + a * 128
                                                  : hh * QB + (a + 1) * 128],
                                        p_ab[:, hh * QB + b * 128
                                             : hh * QB + (b + 1) * 128],
                                    )
                        else:
                            nc.scalar.activation(
                                p_ab[:], s_ab[:],
                                mybir.ActivationFunctionType.Exp, scale=SCALE,
                            )
                        cur = (gs, p_ab)
                if cur is not None:
                    pipe.append(cur)
                tfin = gs - 2
                if tfin in OFFLOAD:
                    # bit-trick exp, step 2: P = S(B1) + S(B1-64); the pair
                    # cancels the interpolation ripple to ~0.6% rms and
                    # round(t-64) == round(t)-64 exactly -> int16 subtract
                    y1 = y1s.pop(tfin)
                    y2 = y_pool.tile([128, 2 * QB], mybir.dt.int16,
                                     tag="y2", name=f"y2_{tfin}")
                    nc.vector.tensor_scalar_sub(y2[:], y1[:], 64)
                    p_fin = p_pool.tile([128, 2 * QB], BF16, tag="p",
                                        name=f"pf{tfin}")
                    nc.vector.tensor_add(
                        p_fin[:], y2[:].bitcast(BF16), y1[:].bitcast(BF16)
                    )
                    for pi, (pgs, pv) in enumerate(pipe):
                        if pgs == tfin:
                            pipe[pi] = (pgs, p_fin)
                            break
                prevs = []
                while pipe and len(prevs) < 3:
                    hkb = pipe[0][0] % KBLKS
                    need = OFF_SKEW if pipe[0][0] in OFFLOAD else (
                        max(SKEW, 18 - hkb) if (DIAG_ON and hkb >= KBLKS - QSUB) else SKEW
                    )
                    if gs - pipe[0][0] >= need or gs >= NSTEP + SKEW - 1:
                        prevs.append(pipe.pop(0))
                    else:
                        break
                for prev in prevs:
                    gsp, pp = prev
                    prp, kbp = divmod(gsp, KBLKS)
                    if kbp == 0:
                        ctx_a = ctx_psum.tile([128, QSUB * AUG], F32, tag="ctx",
                                              name=f"ctxa{prp}")
                        ctx_b = ctx_psum.tile([128, QSUB * AUG], F32, tag="ctx",
                                              name=f"ctxb{prp}")
                        tiles[prp][3] = ctx_a
                        tiles[prp][4] = ctx_b
                    _, _, xa_t, ctx_a, ctx_b = tiles[prp]
                    for j, ctx_t in ((0, ctx_a), (1, ctx_b)):
                        asl = slice(j * KBLKS * AUG + kbp * AUG,
                                    j * KBLKS * AUG + (kbp + 1) * AUG)
                        for qs in range(QSUB):
                            # The 4 qs accumulation groups share one psum
                            # bank (= one 2KB zero region): only the tile's
                            # first matmul may set start (start marks the
                            # WHOLE region pending-zero, so later groups'
                            # first writes still overwrite-not-accumulate),
                            # and only the last sets stop.
                            nc.tensor.matmul(
                                ctx_t[:, qs * AUG : (qs + 1) * AUG],
                                pp[:, j * QB + qs * 128 : j * QB + (qs + 1) * 128],
                                xa_t[:, asl],
                                start=(kbp == 0 and qs == 0),
                                stop=(kbp == KBLKS - 1 and qs == QSUB - 1),
                            )
                    g = plan.get(gsp)
                    if g is not None:
                        with tc.tile_wait_until(floor_ms(gsp)):
                            emit_proj_group(g[0], g[1], g[2], prj_psum, "prj")
                    if kbp == KBLKS - 1:
                        emit_drain(prp, ctx_a, ctx_b)
    nc.compile()
    return nc


def _to_bf16(x):
    import ml_dtypes

    return np.asarray(x, np.float32).astype(ml_dtypes.bfloat16)


def shard_inputs(key, W_ctx, b_ctx):
    """Host-side prep of per-core input dicts (bf16 layouts).

    Keys are permuted per core so the core's own query block comes first:
    softmax is key-order invariant, and this puts the symmetric diagonal
    score block at kb 0..3 identically on every core (SPMD-friendly)."""
    Bv = key.shape[0]
    cores_per_batch = NCORES // Bv

    key = np.asarray(key, dtype=np.float32)
    wt_host = np.ascontiguousarray(
        np.asarray(W_ctx, np.float32).T.reshape(HC, 128, H).transpose(1, 0, 2)
        .reshape(128, HC * H)
    )
    bias_host = np.asarray(b_ctx, np.float32).reshape(1, H)
    wt_b = _to_bf16(wt_host)
    bias_b = _to_bf16(bias_host)
    ones_b = _to_bf16(np.ones((1, 128), dtype=np.float32))
    ident_b = _to_bf16(np.eye(128, dtype=np.float32))

    in_maps = []
    meta = []
    for c in range(NCORES):
        b = c // cores_per_batch
        qlo = (c % cores_per_batch) * QB
        xp = np.concatenate(
            [key[b, :qlo], key[b, qlo + QB :], key[b, qlo : qlo + QB]], axis=0
        )
        xh = xp.reshape(L, NH, DH)
        # xt: [NPAIR, 128, L]; pair p rows 0:64 = head 2p, 64:128 = head 2p+1
        xt_full = np.ascontiguousarray(
            xh.transpose(1, 2, 0).reshape(NPAIR, 2 * DH, L)
        )
        # xa: [NH, 128, KBLKS*AUG] with ones at kb*AUG+DH, then pair-merged
        xa_full = np.empty((NH, 128, KBLKS * AUG), dtype=np.float32)
        xa_view = xa_full.reshape(NH, 128, KBLKS, AUG)
        xa_view[..., DH] = 1.0
        xa_view[..., 0:DH] = xh.reshape(KBLKS, 128, NH, DH).transpose(2, 1, 0, 3)
        ka = KBLKS * AUG
        xa_pair = np.ascontiguousarray(
            xa_full.reshape(NPAIR, 2, 128, ka).transpose(0, 2, 1, 3)
            .reshape(NPAIR, 128, 2 * ka)
        )
        xt_b = _to_bf16(xt_full)
        in_maps.append(
            {
                "xt": xt_b,
                "xq": np.ascontiguousarray(xt_b[:, :, L - QB : L]),
                "xa": _to_bf16(xa_pair),
                "wt": wt_b,
                "bias": bias_b,
                "ones": ones_b,
                "ident": ident_b,
            }
        )
        meta.append((b, qlo))
    return in_maps, meta


_NC_CACHE = {}


def kernel(key, W_ctx, b_ctx):
    from concourse.bass_utils import run_bass_kernel_spmd

    key = np.asarray(key, dtype=np.float32)
    if "nc" not in _NC_CACHE:
        _NC_CACHE["nc"] = build_nc()
    nc = _NC_CACHE["nc"]
    in_maps, meta = shard_inputs(key, W_ctx, b_ctx)
    res = run_bass_kernel_spmd(nc, in_maps, list(range(NCORES)))
    outf = np.empty((B, L, H), dtype=np.float32)
    for c, (b, qlo) in enumerate(meta):
        outf[b, qlo : qlo + QB] = res.results[c]["out"]
    return outf


# revision 31
# speedup vs baseline: 1.3361x; 1.0020x over previous
"""Multi-head self-attention (QK^T -> softmax -> ctx -> linear) on 8 TRN2 cores.

Sharding: each core owns one (batch, query-block) shard: batch = core//4,
queries [qlo, qlo+512) with qlo = (core%4)*512. Attention needs all keys of
the core's batch, so keys are replicated per batch; no collectives needed.

Per core (head h, its 512 queries q, all 2048 keys k), all operands bf16:
  S_T[k, q]  = sum_d x[k, hd+d] * x[q, hd+d]            (PE, psum f32)
  P_T[k, q]  = exp(0.125 * S_T[k, q])                   (ACT, psum->sbuf bf16)
  ctx[q, m]  = sum_k P_T[k, q] * xa[k, m]               (PE; P parked as the
               stationary operand so only the 65-wide xa side streams;
               m = 64 dims + ones column -> softmax denominator at m=64)
  chunk[q,i] = ctx[q, d] / ctx[q, 64]                   (DVE tensor_scalar,
               per-partition recip scalar; -> bf16)
  chunkT     = transpose(chunk)                         (DMA xbar transpose)
  out[q, o]  = sum_i chunkT[i, q] * W[o, i] + b[o]      (PE + DVE adds)

The PE cost model charges only streamed output columns (stationary loads are
free), so parking P halves ctx cost vs streaming it; everything else is
orientation-chosen to keep output partitions full.
"""

import sys

for _p in ("/opt/trn_rl_repo", "/root/.axon_site/_ro/trn_rl_repo"):
    if _p not in sys.path:
        sys.path.append(_p)

import numpy as np

import concourse.bacc as bacc
import concourse.bass as bass
import concourse.mybir as mybir
import concourse.tile as tile

F32 = mybir.dt.float32
BF16 = mybir.dt.bfloat16

B, L, H, NH, DH = 2, 2048, 1024, 16, 64
NCORES = 8
QB = 512
KBLKS = L // 128          # 16 key blocks
NPAIR = NH // 2           # 8 head pairs
AUG = DH + 1              # 64 dims + ones column
HC = H // 128             # 8 hidden chunks (one per pair)
OBW = 512                 # proj output column block
SCALE = float(1.0 / np.sqrt(DH))
QSUB = QB // 128          # 4 query subtiles


def build_nc():
    nc = bacc.Bacc("TRN2")
    xt = nc.declare_dram_parameter("xt", [NPAIR, 128, L], BF16, isOutput=False)
    xq = nc.declare_dram_parameter("xq", [NPAIR, 128, QB], BF16, isOutput=False)
    xa = nc.declare_dram_parameter("xa", [NPAIR, 128, 2 * KBLKS * AUG], BF16, isOutput=False)
    wt = nc.declare_dram_parameter("wt", [128, HC * H], BF16, isOutput=False)
    bias = nc.declare_dram_parameter("bias", [1, H], BF16, isOutput=False)
    ones = nc.declare_dram_parameter("ones", [1, 128], BF16, isOutput=False)
    ident = nc.declare_dram_parameter("ident", [128, 128], BF16, isOutput=False)
    out = nc.declare_dram_parameter("out", [QB, H], F32, isOutput=True)

    NSTEP = NPAIR * KBLKS
    SKEW = 2
    # scheduling floor per global step (ns): keeps the tile scheduler from
    # hoisting proj work into much earlier PE positions, where an unmet
    # transpose dep would stall the PE counter that gates the exp stream.
    TSTEP_NS = 1040.0
    TBASE_NS = 4600.0

    def floor_ms(gs):
        return (TBASE_NS + gs * TSTEP_NS) / 1e6

    # Schraudolph-pair constants (bf16 domain): t = scores*SCALE*log2(e);
    # int16 = t*128 + 127*128 + C, with C folding the /2 normalization
    EXPA = float(128.0 * SCALE * 1.4426950408889634)
    # +22.375: scale calibration so the equal-weight pair averages to 1.0x
    # of true exp (matching the ACT-exp key blocks in the same softmax)
    EXPB1 = float(127 * 128 - 128 + 22.375)
    EXPB2 = float(127 * 128 - 192 + 22.375)
    # offload in runs of 3 so the psum-slot refill chain is amortized once
    # per run instead of once per step
    OFF_STARTS = globals().get('_OFF_STARTS', ())
    DIAG_ON = bool(globals().get('_DIAG_ON', False))
    OFFLOAD = frozenset(
        g for start in OFF_STARTS for g in (start, start + 1, start + 2)
    )
    OFF_SKEW = 5

    with tile.TileContext(nc) as tc:
        with (
            tc.tile_pool(name="xt", bufs=2) as xt_pool,
            tc.tile_pool(name="xq", bufs=2) as xq_pool,
            tc.tile_pool(name="xa", bufs=2) as xa_pool,
            tc.tile_pool(name="p", bufs=10) as p_pool,
            tc.tile_pool(name="cq", bufs=8) as cq_pool,
            tc.tile_pool(name="rc", bufs=4) as rc_pool,
            tc.tile_pool(name="y", bufs=4) as y_pool,
            tc.tile_pool(name="consts", bufs=1) as consts,
            tc.tile_pool(name="spsum", bufs=2, space="PSUM") as s_psum,
            tc.tile_pool(name="ctxpsum", bufs=3, space="PSUM") as ctx_psum,
            tc.tile_pool(name="prjpsum", bufs=1, space="PSUM") as prj_psum,
        ):
            ones_t = consts.tile([1, 128], BF16)
            ident_t = consts.tile([128, 128], BF16)
            bias_t = consts.tile([1, H], BF16)
            bias_bc = consts.tile([128, H], F32)

            wt_ts = [
                consts.tile([128, H], BF16, tag=f"wt{c}", name=f"wt{c}")
                for c in range(HC)
            ]
            chunks = [
                consts.tile([128, QB], BF16, tag=f"ch{c}", name=f"ch{c}")
                for c in range(HC)
            ]
            acc = [
                consts.tile([128, H], F32, tag=f"acc{q}", name=f"acc{q}")
                for q in range(QSUB)
            ]

            def pair_dmas(pr, split_xt=False):
                xt_t = xt_pool.tile([128, L], BF16, tag="xt", name=f"xt{pr}")
                xq_t = xq_pool.tile([128, QB], BF16, tag="xq", name=f"xq{pr}")
                nc.sync.dma_start(xq_t[:], xq[pr])
                if split_xt:
                    # first key quarter lands fast so scores can start early
                    nc.sync.dma_start(xt_t[:, 0:512], xt[pr][:, 0:512])
                    nc.sync.dma_start(xt_t[:, 512:L], xt[pr][:, 512:L])
                else:
                    nc.sync.dma_start(xt_t[:], xt[pr])
                xa_t = xa_pool.tile([128, 2 * KBLKS * AUG], BF16, tag="xa", name=f"xa{pr}")
                nc.sync.dma_start(xa_t[:], xa[pr])
                return [xt_t, xq_t, xa_t, None, None]

            def emit_proj_group(clist, qs, ob, pool, tag, eng=None):
                obsl = slice(ob * OBW, (ob + 1) * OBW)
                qsl = slice(qs * 128, (qs + 1) * 128)
                cp = pool.tile([128, 2 * QB] if tag == "s" else [128, OBW],
                               F32, tag=tag, name=f"cp{clist[0]}_{qs}_{ob}")
                for i, c in enumerate(clist):
                    nc.tensor.matmul(
                        cp[:, 0:OBW], chunks[c][:, qsl], wt_ts[c][:, obsl],
                        start=(i == 0), stop=(i == len(clist) - 1),
                    )
                prev = bias_bc if clist[0] == 0 else acc[qs]
                (eng or nc.vector).tensor_add(
                    acc[qs][:, obsl], cp[:, 0:OBW], prev[:, obsl]
                )

            def emit_drain(prp, ctx_a, ctx_b):
                # normalize pair prp's ctx accumulators and transpose into
                # the proj chunk tile; for the final pair, pipeline the
                # remaining projection groups and output DMAs per q-subtile
                last = prp == NPAIR - 1
                rc_a = rc_pool.tile([128, QSUB], F32, tag="rc", name=f"rca{prp}")
                rc_b = rc_pool.tile([128, QSUB], F32, tag="rc", name=f"rcb{prp}")
                with nc.allow_low_precision(reason="softmax denominator recip"):
                    nc.vector.reciprocal(rc_a[:], ctx_a[:, DH : QSUB * AUG : AUG])
                    nc.vector.reciprocal(rc_b[:], ctx_b[:, DH : QSUB * AUG : AUG])
                for qs in range(QSUB):
                    cq = cq_pool.tile([128, 128], BF16, tag="cq", name=f"cq{prp}_{qs}")
                    nc.vector.tensor_scalar_mul(
                        cq[:, 0:DH], ctx_a[:, qs * AUG : qs * AUG + DH],
                        rc_a[:, qs : qs + 1],
                    )
                    nc.vector.tensor_scalar_mul(
                        cq[:, DH : 2 * DH], ctx_b[:, qs * AUG : qs * AUG + DH],
                        rc_b[:, qs : qs + 1],
                    )
                    if not last:
                        nc.sync.dma_start_transpose(
                            chunks[prp][:, qs * 128 : (qs + 1) * 128], cq[:, :]
                        )
                        continue
                    # final pair: PE-transpose (HWDGE is slow to drain) + ACT
                    # copy; fused (5,6,7) groups follow in a second pass so
                    # the DVE adds overlap later q-subtiles' normalizes
                    tp = s_psum.tile([128, 128], BF16, tag="s", name=f"tp{qs}")
                    nc.tensor.transpose(tp[:], cq[:, :], ident_t[:])
                    qsl = slice(qs * 128, (qs + 1) * 128)
                    nc.scalar.copy(chunks[prp][:, qsl], tp[:])
                if last:
                    for qs in range(QSUB):
                        qsl = slice(qs * 128, (qs + 1) * 128)
                        clist = (5, 6, 7) if qs == 0 else (7,)
                        cp = s_psum.tile([128, 2 * QB], F32, tag="s", name=f"tcp{qs}")
                        for ob in range(2):
                            obsl = slice(ob * OBW, (ob + 1) * OBW)
                            for i, c in enumerate(clist):
                                nc.tensor.matmul(
                                    cp[:, obsl], chunks[c][:, qsl], wt_ts[c][:, obsl],
                                    start=(i == 0), stop=(i == len(clist) - 1),
                                )
                        if qs < QSUB - 1:
                            nc.vector.tensor_add(acc[qs][:], cp[:], acc[qs][:])
                            nc.sync.dma_start(out[qsl, :], acc[qs][:])
                        else:
                            # quarter-split so the final DMA transfer is small
                            for q4 in range(4):
                                sl = slice(q4 * 256, (q4 + 1) * 256)
                                nc.vector.tensor_add(
                                    acc[qs][:, sl], cp[:, sl], acc[qs][:, sl]
                                )
                                nc.sync.dma_start(out[qsl, sl], acc[qs][:, sl])

            # proj emission plan: step -> (chunk list, qs, ob)
            # chunk-pairs (0,1)@pairs2-3, (2,3)@pairs4-5; chunk 4 singles@pair6,
            # (5,6)@pair7, chunk 7 + leftovers in the tail.
            # window steps start ~4 kbs after the newer chunk's drain so the
            # first group never waits on an in-flight transpose
            plan = {}
            for hp, base_pr in ((0, 2), (1, 4)):
                cl = [2 * hp, 2 * hp + 1]
                w1, w2 = globals().get('_WPLAN', ((8, 10, 12, 14), (2, 6, 10, 14)))
                steps = [base_pr * KBLKS + k for k in w1] + [
                    (base_pr + 1) * KBLKS + k for k in w2
                ]
                for gidx, st in enumerate(steps):
                    qs, ob = divmod(gidx, 2)
                    plan[st] = (cl, qs, ob)
            for i, kbw in enumerate((3, 5, 7, 9, 11, 13, 15)):
                plan[6 * KBLKS + kbw] = ([4], i // 2, i % 2)
            plan[7 * KBLKS + 1] = ([4], 3, 1)
            for i, kbw in enumerate((5, 7, 9, 11, 13, 15)):
                plan[7 * KBLKS + kbw] = ([5, 6], 1 + i // 2, i % 2)

            tiles = {}
            pipe = []
            y1s = {}
            diag_p = {}
            for gs in range(NSTEP + OFF_SKEW):
                cur = None
                if gs < NSTEP:
                    pr, kb = divmod(gs, KBLKS)
                    if pr == 0 and kb == 0:
                        warm = consts.tile([1, 512], BF16)
                        nc.vector.memset(warm[:], 1.0)
                        for wi in range(int(globals().get('_WARMN', 6))):
                            wps = prj_psum.tile([128, OBW], F32, tag="prj",
                                                name=f"warm{wi}")
                            nc.tensor.matmul(
                                wps[:], warm[0:1, 0:128], warm[0:1, :],
                                start=True, stop=True,
                            )
                        tiles[0] = pair_dmas(0, split_xt=True)
                        nc.sync.dma_start(wt_ts[0][:], wt[:, 0:H])
                        nc.sync.dma_start(ones_t[:], ones[:])
                        nc.sync.dma_start(bias_t[:], bias[:])
                        nc.sync.dma_start(ident_t[:], ident[:])
                    if pr == 0 and kb == 6:
                        # bias broadcast across partitions via K=1 ones matmul
                        # (uses the proj psum bank, idle until pair 2, so the
                        # late-landing bias DMA never stalls the score slots)
                        for ob in range(H // OBW):
                            obsl = slice(ob * OBW, (ob + 1) * OBW)
                            bps = prj_psum.tile([128, OBW], F32, tag="prj")
                            nc.tensor.matmul(
                                bps[:], ones_t[0:1, :], bias_t[0:1, obsl],
                                start=True, stop=True,
                            )
                            nc.vector.tensor_copy(bias_bc[:, obsl], bps[:])
                    if kb == 2 and pr + 1 < NPAIR:
                        # floor keeps prefetches behind the previous
                        # drain's transposes in the HWDGE queue
                        with tc.tile_wait_until(floor_ms((pr + 1) * KBLKS - 6)):
                            tiles[pr + 1] = pair_dmas(pr + 1)
                    if kb == 4 and pr + 1 < NPAIR:
                        with tc.tile_wait_until(floor_ms(min((pr + 3) * KBLKS - 10, NSTEP))):
                            nc.sync.dma_start(
                                wt_ts[pr + 1][:], wt[:, (pr + 1) * H : (pr + 2) * H]
                            )
                    xt_t, xq_t, xa_t, _, _ = tiles[pr]
                    ksl = slice(kb * 128, (kb + 1) * 128)
                    # keys are permuted per core so kb 12-15 is the core's
                    # own query block: S(i,j) = S(j,i)^T there, so compute
                    # only q-subtiles b <= kb-12 and fill the rest by
                    # transposing later steps' exp'd blocks (exact, on the
                    # otherwise idle DMA engines)
                    dw = (kb - 11) * 128 if (DIAG_ON and kb >= KBLKS - QSUB) else QB
                    s_ab = s_psum.tile([128, 2 * QB], F32, tag="s")
                    nc.tensor.matmul(
                        s_ab[:, 0:dw], xt_t[0:64, ksl], xq_t[0:64, 0:dw],
                        start=True, stop=True,
                    )
                    nc.tensor.matmul(
                        s_ab[:, QB : QB + dw], xt_t[64:128, ksl],
                        xq_t[64:128, 0:dw],
                        start=True, stop=True,
                    )
                    if gs in OFFLOAD:
                        # bit-trick exp, step 1 of 2: y1 = int16(s*A + B1).
                        # This is the only psum reader, so the score slot is
                        # released at ACT-parity latency; the combine runs two
                        # steps later (see the finish block below).
                        y1 = y_pool.tile([128, 2 * QB], mybir.dt.int16,
                                         tag="y1", name=f"y1_{gs}")
                        nc.vector.tensor_scalar(
                            y1[:], s_ab[:], EXPA, EXPB1,
                            op0=mybir.AluOpType.mult, op1=mybir.AluOpType.add,
                        )
                        y1s[gs] = y1
                        cur = (gs, None)
                    else:
                        p_ab = p_pool.tile([128, 2 * QB], BF16, tag="p")
                        if DIAG_ON and kb >= KBLKS - QSUB:
                            a = kb - (KBLKS - QSUB)
                            sv = s_ab.rearrange("p (h q) -> p h q", h=2)
                            pv = p_ab.rearrange("p (h q) -> p h q", h=2)
                            nc.scalar.activation(
                                pv[:, :, 0:dw], sv[:, :, 0:dw],
                                mybir.ActivationFunctionType.Exp, scale=SCALE,
                            )
                            diag_p[a] = p_ab
                            for b in range(a):
                                for hh in range(2):
                                    nc.sync.dma_start_transpose(
                                        diag_p[b][:, hh * QB + a * 128
                                                  : hh * QB + (a + 1) * 128],
                                        p_ab[:, hh * QB + b * 128
                                             : hh * QB + (b + 1) * 128],
                                    )
                        else:
                            nc.scalar.activation(
                                p_ab[:], s_ab[:],
                                mybir.ActivationFunctionType.Exp, scale=SCALE,
                            )
                        cur = (gs, p_ab)
                if cur is not None:
                    pipe.append(cur)
                tfin = gs - 2
                if tfin in OFFLOAD:
                    # bit-trick exp, step 2: P = S(B1) + S(B1-64); the pair
                    # cancels the interpolation ripple to ~0.6% rms and
                    # round(t-64) == round(t)-64 exactly -> int16 subtract
                    y1 = y1s.pop(tfin)
                    y2 = y_pool.tile([128, 2 * QB], mybir.dt.int16,
                                     tag="y2", name=f"y2_{tfin}")
                    nc.vector.tensor_scalar_sub(y2[:], y1[:], 64)
                    p_fin = p_pool.tile([128, 2 * QB], BF16, tag="p",
                                        name=f"pf{tfin}")
                    nc.vector.tensor_add(
                        p_fin[:], y2[:].bitcast(BF16), y1[:].bitcast(BF16)
                    )
                    for pi, (pgs, pv) in enumerate(pipe):
                        if pgs == tfin:
                            pipe[pi] = (pgs, p_fin)
                            break
                prevs = []
                while pipe and len(prevs) < 3:
                    hkb = pipe[0][0] % KBLKS
                    need = OFF_SKEW if pipe[0][0] in OFFLOAD else (
                        max(SKEW, 18 - hkb) if (DIAG_ON and hkb >= KBLKS - QSUB) else SKEW
                    )
                    if gs - pipe[0][0] >= need or gs >= NSTEP + SKEW - 1:
                        prevs.append(pipe.pop(0))
                    else:
                        break
                for prev in prevs:
                    gsp, pp = prev
                    prp, kbp = divmod(gsp, KBLKS)
                    if kbp == 0:
                        ctx_a = ctx_psum.tile([128, QSUB * AUG], F32, tag="ctx",
                                              name=f"ctxa{prp}")
                        ctx_b = ctx_psum.tile([128, QSUB * AUG], F32, tag="ctx",
                                              name=f"ctxb{prp}")
                        tiles[prp][3] = ctx_a
                        tiles[prp][4] = ctx_b
                    _, _, xa_t, ctx_a, ctx_b = tiles[prp]
                    for j, ctx_t in ((0, ctx_a), (1, ctx_b)):
                        asl = slice(j * KBLKS * AUG + kbp * AUG,
                                    j * KBLKS * AUG + (kbp + 1) * AUG)
                        for qs in range(QSUB):
                            # The 4 qs accumulation groups share one psum
                            # bank (= one 2KB zero region): only the tile's
                            # first matmul may set start (start marks the
                            # WHOLE region pending-zero, so later groups'
                            # first writes still overwrite-not-accumulate),
                            # and only the last sets stop.
                            nc.tensor.matmul(
                                ctx_t[:, qs * AUG : (qs + 1) * AUG],
                                pp[:, j * QB + qs * 128 : j * QB + (qs + 1) * 128],
                                xa_t[:, asl],
                                start=(kbp == 0 and qs == 0),
                                stop=(kbp == KBLKS - 1 and qs == QSUB - 1),
                            )
                    g = plan.get(gsp)
                    if g is not None:
                        with tc.tile_wait_until(floor_ms(gsp)):
                            emit_proj_group(g[0], g[1], g[2], prj_psum, "prj")
                    if kbp == KBLKS - 1:
                        emit_drain(prp, ctx_a, ctx_b)
    nc.compile()
    return nc


def _to_bf16(x):
    import ml_dtypes

    return np.asarray(x, np.float32).astype(ml_dtypes.bfloat16)


def shard_inputs(key, W_ctx, b_ctx):
    """Host-side prep of per-core input dicts (bf16 layouts).

    Keys are permuted per core so the core's own query block comes first:
    softmax is key-order invariant, and this puts the symmetric diagonal
    score block at kb 0..3 identically on every core (SPMD-friendly)."""
    Bv = key.shape[0]
    cores_per_batch = NCORES // Bv

    key = np.asarray(key, dtype=np.float32)
    wt_host = np.ascontiguousarray(
        np.asarray(W_ctx, np.float32).T.reshape(HC, 128, H).transpose(1, 0, 2)
        .reshape(128, HC * H)
    )
    bias_host = np.asarray(b_ctx, np.float32).reshape(1, H)
    wt_b = _to_bf16(wt_host)
    bias_b = _to_bf16(bias_host)
    ones_b = _to_bf16(np.ones((1, 128), dtype=np.float32))
    ident_b = _to_bf16(np.eye(128, dtype=np.float32))

    in_maps = []
    meta = []
    for c in range(NCORES):
        b = c // cores_per_batch
        qlo = (c % cores_per_batch) * QB
        xp = np.concatenate(
            [key[b, :qlo], key[b, qlo + QB :], key[b, qlo : qlo + QB]], axis=0
        )
        xh = xp.reshape(L, NH, DH)
        # xt: [NPAIR, 128, L]; pair p rows 0:64 = head 2p, 64:128 = head 2p+1
        xt_full = np.ascontiguousarray(
            xh.transpose(1, 2, 0).reshape(NPAIR, 2 * DH, L)
        )
        # xa: [NH, 128, KBLKS*AUG] with ones at kb*AUG+DH, then pair-merged
        xa_full = np.empty((NH, 128, KBLKS * AUG), dtype=np.float32)
        xa_view = xa_full.reshape(NH, 128, KBLKS, AUG)
        xa_view[..., DH] = 1.0
        xa_view[..., 0:DH] = xh.reshape(KBLKS, 128, NH, DH).transpose(2, 1, 0, 3)
        ka = KBLKS * AUG
        xa_pair = np.ascontiguousarray(
            xa_full.reshape(NPAIR, 2, 128, ka).transpose(0, 2, 1, 3)
            .reshape(NPAIR, 128, 2 * ka)
        )
        xt_b = _to_bf16(xt_full)
        in_maps.append(
            {
                "xt": xt_b,
                "xq": np.ascontiguousarray(xt_b[:, :, L - QB : L]),
                "xa": _to_bf16(xa_pair),
                "wt": wt_b,
                "bias": bias_b,
                "ones": ones_b,
                "ident": ident_b,
            }
        )
        meta.append((b, qlo))
    return in_maps, meta


_NC_CACHE = {}


def kernel(key, W_ctx, b_ctx):
    from concourse.bass_utils import run_bass_kernel_spmd

    key = np.asarray(key, dtype=np.float32)
    if "nc" not in _NC_CACHE:
        _NC_CACHE["nc"] = build_nc()
    nc = _NC_CACHE["nc"]
    in_maps, meta = shard_inputs(key, W_ctx, b_ctx)
    res = run_bass_kernel_spmd(nc, in_maps, list(range(NCORES)))
    outf = np.empty((B, L, H), dtype=np.float32)
    for c, (b, qlo) in enumerate(meta):
        outf[b, qlo : qlo + QB] = res.results[c]["out"]
    return outf
